# revision 55
# baseline (speedup 1.0000x reference)
"""Trainium2 Bass kernel for the MoE routing layer (nn_MoELayer_20358144983731).

Strategy
--------
Routing depends only on the atom's type (32 types), and with top-2-of-8
routing each atom needs exactly 3 expert MLPs (2 routed + 1 shared) instead
of the reference's dense 9.  The gate is tiny, so it is computed on the host;
atoms are sorted by type and packed into fixed-capacity slots (CAP=2048
atoms; types larger than CAP get a small spill slot), distributed across the
8 NeuronCores.  Every atom of a type shares the same two routed experts and
scalar routing weights, so the whole device program is data-driven (weights /
biases / scales arrive as per-core input tensors) and a single SPMD program
runs on all 8 cores.

Per slot the device computes, transposed (z.T = [dout, atoms]) so the
dout-dim bias lands on partitions:
    y = w0*tanh(X W0 + b0) + w1*tanh(X W1 + b1) + tanh(X Ws + bs)

The kernel is ScalarE-bound (tanh ACT is scalar-engine-only, 1 elem/cyc/lane
plus ~650ns access-latency per instruction), so the structure keeps the
scalar queue saturated with nothing but back-to-back ACTs:
  - a dummy warm-up ACT triggers the tanh table load during the DMA phase
  - DMA is packet-dispatch-bound (~one descriptor per partition row per
    transfer), so the ramp minimizes wave-0 transfers across the three DMA
    rings (scalar/sync/gpsimd); big0's xt arrives as half-tiles and its
    first ACT is split in halves so the big stream starts as early as
    possible, while later xt/weight transfers are deferred into the slot
    loop where they queue behind output-DMA waits
  - one spill slot runs first (small ACTs warm the pipe while the big xt
    transfers land); remaining spill slots run last with their expert bias
    pre-folded into the spill atoms (x~ = x + b W^-1, solved on the host),
    which lets the 3 streams share one bias-free ACT and keeps the tail short
  - matmuls run in bf16 (weights stationary, atoms moving, N=512, fp32 PSUM
    accumulation, 4-bank tiles double buffered); the shared-expert weights
    load once and serve every slot's stream-2 matmuls
  - the 3-stream combine is two scalar_tensor_tensor ops on the vector
    engine (bf16 in / fp32 out); the final slot's last ACT and combine are
    split in halves and its output partitioned over the then-idle DMA rings
    so the tail drains fast
"""

import sys

import numpy as np

try:
    import concourse  # noqa: F401
except ImportError:  # grading container path
    sys.path.insert(0, "/opt/trn_rl_repo")

import ml_dtypes

import concourse.bacc as bacc
import concourse.mybir as mybir
import concourse.tile as tile
from concourse.bass_utils import run_bass_kernel_spmd

NB, NLOC = 4, 16384
DIN, DOUT = 256, 256
NTYPES = 32
N_CORES = 8
NS = 3  # streams: routed expert 0, routed expert 1, shared expert
CAP = 2048  # big-slot capacity (4 PSUM banks at fp32)
BF16 = ml_dtypes.bfloat16
WCOL = 2 * 2 * 2 * 128  # routed weight columns per slot (2 streams)

_compiled_cache = {}


def _build_nc(nbig, nspill, sl):
    """Build + compile the SPMD Tile kernel.

    nbig:   number of CAP-length slots per core
    nspill: number of spill slots per core (0 = none); the first runs
            before the big slots (bias via ACT), the rest run after them
            (bias pre-folded into x, single ACT per dout-half)
    sl:     spill slot length (multiple of 128)
    """
    f32 = mybir.dt.float32
    bf16 = mybir.dt.bfloat16
    Tanh = mybir.ActivationFunctionType.Tanh
    mult = mybir.AluOpType.mult
    add = mybir.AluOpType.add

    nslots = nbig + nspill
    nback = max(nspill - 1, 0)

    nc = bacc.Bacc("TRN2", target_bir_lowering=False, debug=False)
    xtb_d = nc.dram_tensor("xtb", [nbig * 128, 2 * CAP], bf16, kind="ExternalInput")
    if nspill:
        xtsf_d = nc.dram_tensor("xtsf", [128, 2 * sl], bf16, kind="ExternalInput")
    if nback:
        xtsb_d = nc.dram_tensor(
            "xtsb", [nback * 128, NS * 2 * sl], bf16, kind="ExternalInput"
        )
    wsh_d = nc.dram_tensor("wsh", [128, 512], bf16, kind="ExternalInput")
    w_d = nc.dram_tensor("w", [128, nslots * WCOL], bf16, kind="ExternalInput")
    bs_d = nc.dram_tensor("bs", [128, nslots * 8], f32, kind="ExternalInput")
    yb_d = nc.dram_tensor("yb", [nbig * 2 * 128, CAP], f32, kind="ExternalOutput")
    if nspill:
        ys_d = nc.dram_tensor("ys", [nspill * 2 * 128, sl], f32, kind="ExternalOutput")

    with tile.TileContext(nc) as tc:
        with (
            tc.tile_pool(name="const", bufs=1) as constp,
            tc.tile_pool(name="t", bufs=3) as tp,
            tc.tile_pool(name="y", bufs=3) as yp,
            tc.tile_pool(name="ps", bufs=2, space="PSUM") as psp,
        ):
            # dummy ACT so the tanh table load happens during the DMA phase
            dm_in = constp.tile([128, 1], f32, name="dmi")
            dm_out = constp.tile([128, 1], f32, name="dmo")
            nc.vector.memset(dm_in, 0.0)
            nc.scalar.activation(dm_out, dm_in, Tanh, bias=dm_in, scale=1.0)

            # ---- DMA issue: spread across the three DMA-capable queues -
            # scalar (idle until first ACT): wsh, xtb0
            # sync: spill-front xt, b, s, xtb1..3, spill-back xt; later y (c=1)
            # gpsimd: all slot weights; later y (c=0)
            # big0's xt comes as two half-tiles (k=0 / k=1), each striped
            # over two DMA channels, so its matmuls can start ~11us in;
            # other big slots use one whole tile each on the sync channel
            xt0_k = [
                constp.tile([128, CAP], bf16, name=f"xt0k{k}") for k in range(2)
            ]
            xt_big = [None] + [
                constp.tile([128, 2 * CAP], bf16, name=f"xtb{i}")
                for i in range(1, nbig)
            ]
            if nspill:
                xt_f = constp.tile([128, 2 * sl], bf16, name="xtf")
                nc.sync.dma_start(out=xt_f, in_=xtsf_d.ap())
            wsh = constp.tile([128, 512], bf16, name="wsh")
            nc.scalar.dma_start(out=wsh, in_=wsh_d.ap())
            bs_sb = constp.tile([128, nslots * 8], f32)
            nc.sync.dma_start(out=bs_sb, in_=bs_d.ap())
            b_sb = bs_sb[:, : nslots * NS * 2]
            s_sb = bs_sb[:, nslots * NS * 2 :]
            w_sl = {
                si: constp.tile([128, WCOL], bf16, name=f"w{si}")
                for si in range(nslots)
            }

            def w_dma(engine, si):
                engine.dma_start(
                    out=w_sl[si], in_=w_d.ap()[:, si * WCOL : (si + 1) * WCOL]
                )

            # DMA is packet-dispatch-bound (~1 descriptor per partition row),
            # so the ramp is bounded by the number of wave-0 transfers; big0's
            # xt comes as two half-tiles (one DMA each) and everything not
            # needed early is deferred into the slot loop so it queues behind
            # output-DMA waits.
            half = CAP // 2
            if nspill:
                w_dma(nc.gpsimd, nbig)  # front spill slot weights
            nc.scalar.dma_start(
                out=xt0_k[0][:, :half], in_=xtb_d.ap()[0:128, 0:half]
            )
            nc.gpsimd.dma_start(
                out=xt0_k[1][:, :half], in_=xtb_d.ap()[0:128, CAP : CAP + half]
            )
            nc.sync.dma_start(
                out=xt0_k[0][:, half:], in_=xtb_d.ap()[0:128, half:CAP]
            )
            nc.sync.dma_start(
                out=xt0_k[1][:, half:], in_=xtb_d.ap()[0:128, CAP + half : 2 * CAP]
            )
            w_dma(nc.gpsimd, 0)
            xt_b = [
                constp.tile([128, NS * 2 * sl], bf16, name=f"xtbk{j}")
                for j in range(nback)
            ]

            # deferred transfers: issued inside the slot loop right after a
            # blocking output DMA so their packets don't compete with the
            # startup burst on the shared DMA engines
            deferred_sync = [
                (lambda i=i: nc.sync.dma_start(
                    out=xt_big[i], in_=xtb_d.ap()[i * 128 : (i + 1) * 128, :]
                )) for i in range(1, nbig)
            ] + [
                (lambda j=j: w_dma(nc.sync, nbig + 1 + j)) for j in range(nback)
            ] + [
                (lambda j=j: nc.sync.dma_start(
                    out=xt_b[j], in_=xtsb_d.ap()[j * 128 : (j + 1) * 128, :]
                )) for j in range(nback)
            ]
            deferred_gp = [
                (lambda i=i: w_dma(nc.gpsimd, i)) for i in range(1, nbig)
            ]

            # ---- slot schedule: spill-front, bigs, spill-backs ---------
            sched = []
            if nspill:
                sched.append(("front", nbig))
            sched += [("big", i) for i in range(nbig)]
            sched += [("back", nbig + 1 + j) for j in range(nback)]

            def combine(si, t_sb, slen, dst_d, yrow, pieces, tail=False,
                        eng=None, last=False):
                eng = eng or nc.vector
                ycomb = yp.tile([128, slen], f32, tag="yc", name="yc")
                for pi, (h0, h1) in enumerate(pieces):
                    ya = yp.tile([128, slen], f32, tag="ya", name="ya")
                    eng.scalar_tensor_tensor(
                        ya[:, : h1 - h0],
                        t_sb[:, h0:h1],
                        s_sb[:, si * 2 : si * 2 + 1],
                        t_sb[:, 2 * slen + h0 : 2 * slen + h1],
                        mult,
                        add,
                    )
                    eng.scalar_tensor_tensor(
                        ycomb[:, h0:h1],
                        t_sb[:, slen + h0 : slen + h1],
                        s_sb[:, si * 2 + 1 : si * 2 + 2],
                        ya[:, : h1 - h0],
                        mult,
                        add,
                    )
                    if last:
                        # very last outputs: halve the packet count per ring
                        # by splitting each piece over two idle rings
                        for q, (r0, r1) in zip(
                            (nc.scalar, nc.gpsimd), ((0, 64), (64, 128))
                        ):
                            q.dma_start(
                                out=dst_d.ap()[yrow + r0 : yrow + r1, h0:h1],
                                in_=ycomb[r0:r1, h0:h1],
                            )
                        continue
                    if tail:
                        # scalar's DMA channel is idle once ACTs are done,
                        # but it is the slow one: give it the earliest piece
                        q = (nc.scalar, nc.gpsimd, nc.sync, nc.gpsimd)[pi % 4]
                    else:
                        q = nc.gpsimd if (yrow // 128) % 2 == 0 else nc.sync
                    q.dma_start(
                        out=dst_d.ap()[yrow : yrow + 128, h0:h1],
                        in_=ycomb[:, h0:h1],
                    )

            for oi, (kind, si) in enumerate(sched):
                is_last_big = kind == "big" and si == nbig - 1
                if kind == "big":
                    slen, xt_sb, dst_d, r0 = CAP, xt_big[si], yb_d, si
                    if si == 0:
                        rhs = lambda k, a0, al: xt0_k[k][:, a0 : a0 + al]
                    else:
                        rhs = (lambda xt: lambda k, a0, al:
                               xt[:, k * CAP + a0 : k * CAP + a0 + al])(xt_sb)
                elif kind == "front":
                    slen, xt_sb, dst_d, r0 = sl, xt_f, ys_d, 0
                    rhs = lambda k, a0, al: xt_f[:, k * sl + a0 : k * sl + a0 + al]
                else:
                    slen, xt_sb, dst_d, r0 = sl, xt_b[si - nbig - 1], ys_d, si - nbig

                for c in range(2):
                    yrow = (r0 * 2 + c) * 128
                    if kind == "back":
                        # 3 streams in one bias-free ACT (bias folded into x)
                        ps = psp.tile([128, NS * slen], f32, tag="ps", name="ps")
                        first = True
                        for s in (2, 0, 1):
                            for k in range(2):
                                if s == 2:
                                    lhsT = wsh[:, (c * 2 + k) * 128 : (c * 2 + k + 1) * 128]
                                else:
                                    blk = (s * 2 + c) * 2 + k
                                    lhsT = w_sl[si][:, blk * 128 : (blk + 1) * 128]
                                nc.tensor.matmul(
                                    ps[:, s * slen : (s + 1) * slen],
                                    lhsT,
                                    xt_sb[:, (s * 2 + k) * slen : (s * 2 + k + 1) * slen],
                                    start=first,
                                    stop=(s == 1 and k == 1),
                                    skip_group_check=True,
                                )
                                first = False
                        t_sb = tp.tile([128, NS * slen], bf16, tag="t", name="t")
                        nc.scalar.activation(t_sb, ps, Tanh, bias=dm_in, scale=1.0)
                        # t layout: stream s at [s*slen,(s+1)*slen) but ACT wrote
                        # psum order (2,0,1)->(0,1,2)? ps col s*slen+a == t col
                        # s*slen+a, so layout already stream-major.
                        combine(si, t_sb, slen, dst_d, yrow, ((0, slen),))
                        continue
                    t_sb = tp.tile([128, NS * slen], bf16, tag="t", name="t")
                    # stream order (2, 0, 1): combine #1 needs t2+t0, so the
                    # tail combine only waits on the last stream's ACT
                    for s in (2, 0, 1):
                        bcol = (si * NS + s) * 2 + c
                        ps = psp.tile([128, slen], f32, tag="ps", name="ps")
                        if kind == "big" and si == 0 and c == 0 and s == 2 \
                                and slen > 512:
                            # first big ACT split in halves: starts once only
                            # half of big0's xt has landed
                            hh = slen // 2
                            for h0 in (0, hh):
                                for k in range(2):
                                    lhsT = wsh[:, (c * 2 + k) * 128 : (c * 2 + k + 1) * 128]
                                    for a0 in range(h0, h0 + hh, 512):
                                        nc.tensor.matmul(
                                            ps[:, a0 : a0 + 512],
                                            lhsT,
                                            rhs(k, a0, 512),
                                            start=(k == 0),
                                            stop=(k == 1),
                                        )
                                nc.scalar.activation(
                                    t_sb[:, s * slen + h0 : s * slen + h0 + hh],
                                    ps[:, h0 : h0 + hh],
                                    Tanh,
                                    bias=b_sb[:, bcol : bcol + 1],
                                    scale=1.0,
                                )
                            continue
                        for k in range(2):
                            if s == 2:
                                lhsT = wsh[:, (c * 2 + k) * 128 : (c * 2 + k + 1) * 128]
                            else:
                                blk = (s * 2 + c) * 2 + k
                                lhsT = w_sl[si][:, blk * 128 : (blk + 1) * 128]
                            for a0 in range(0, slen, 512):
                                al = min(512, slen - a0)
                                nc.tensor.matmul(
                                    ps[:, a0 : a0 + al],
                                    lhsT,
                                    rhs(k, a0, al),
                                    start=(k == 0),
                                    stop=(k == 1),
                                )
                        # tanh + per-partition bias, PSUM -> SBUF (bf16)
                        if s == 1 and is_last_big and c == 1 and slen > 512:
                            # split the final ACT so the combine + output DMA
                            # pipeline starts while its second half runs
                            hh = slen // 2
                            for p0 in (0, hh):
                                nc.scalar.activation(
                                    t_sb[:, slen + p0 : slen + p0 + hh],
                                    ps[:, p0 : p0 + hh],
                                    Tanh,
                                    bias=b_sb[:, bcol : bcol + 1],
                                    scale=1.0,
                                )
                        else:
                            nc.scalar.activation(
                                t_sb[:, s * slen : (s + 1) * slen],
                                ps,
                                Tanh,
                                bias=b_sb[:, bcol : bcol + 1],
                                scale=1.0,
                            )
                    if slen > 512 and is_last_big:
                        hh = slen // 2
                        pieces = ((0, hh), (hh, slen))
                    else:
                        pieces = ((0, slen),)
                    combine(si, t_sb, slen, dst_d, yrow, pieces,
                            tail=is_last_big)
                    if kind == "big":
                        # deferred transfers ride behind the y DMA just
                        # issued on their queue (c=0 -> gpsimd, c=1 -> sync)
                        if c == 0:
                            if deferred_gp:
                                deferred_gp.pop(0)()
                        else:
                            for _ in range(2):
                                if deferred_sync:
                                    deferred_sync.pop(0)()

    nc.compile()
    return nc


def _host_route(type_embeddings, gate_w):
    """Gate on host: per-type top-2 experts + softmax weights (tiny)."""
    logits = type_embeddings.astype(np.float32) @ gate_w.astype(np.float32)
    top2 = np.argsort(-logits, axis=1, kind="stable")[:, :2]
    tv = np.take_along_axis(logits, top2, axis=1)
    e = np.exp(tv - tv.max(axis=1, keepdims=True))
    wts = e / e.sum(axis=1, keepdims=True)
    return top2, wts


def _xt_layout(buf):
    """[nslots, slen, 256] fp32 -> [nslots*128, 2*slen] bf16 with
    row = slot*128 + p, col = k*slen + a, value = buf[slot, a, k*128+p]."""
    ns, slen, _ = buf.shape
    return np.ascontiguousarray(
        buf.reshape(ns, slen, 2, 128).transpose(0, 3, 2, 1)  # [slot, p, k, a]
    ).reshape(ns * 128, 2 * slen).astype(BF16)


def kernel(x, type_embeddings, atom_types, gate_w, expert_w, expert_b,
           shared_w, shared_b, _trace=False, _trace_kwargs=None):
    x = np.asarray(x, dtype=np.float32)
    type_embeddings = np.asarray(type_embeddings, dtype=np.float32)
    atom_types = np.asarray(atom_types)
    gate_w = np.asarray(gate_w, dtype=np.float32)
    expert_w = np.asarray(expert_w, dtype=np.float32)
    expert_b = np.asarray(expert_b, dtype=np.float32)
    shared_w = np.asarray(shared_w, dtype=np.float32)
    shared_b = np.asarray(shared_b, dtype=np.float32)

    top2, wts = _host_route(type_embeddings, gate_w)

    flat_t = atom_types.reshape(-1).astype(np.int64)
    N = flat_t.size
    order = np.argsort(flat_t, kind="stable")
    counts = np.bincount(flat_t, minlength=NTYPES)
    starts = np.zeros(NTYPES + 1, np.int64)
    starts[1:] = np.cumsum(counts)
    xs = x.reshape(N, DIN)[order]

    # pieces: per type a main piece (<= CAP rows) + spill pieces
    # big slot: type t -> core t // (NTYPES//N_CORES), slot t % (..)
    TPC = NTYPES // N_CORES  # big slots per core = 4
    spills = []  # (type, offset_in_type, length)
    for t in range(NTYPES):
        off = CAP
        while off < counts[t]:
            spills.append((t, off, min(CAP, counts[t] - off)))
            off += CAP
    nspill = (len(spills) + N_CORES - 1) // N_CORES
    max_spill = max((ln for _, _, ln in spills), default=0)
    sl = max(((max_spill + 127) // 128) * 128, 128) if nspill else 0
    nback = max(nspill - 1, 0)

    big_buf = np.zeros((N_CORES, TPC, CAP, DIN), np.float32)
    for t in range(NTYPES):
        m = min(int(counts[t]), CAP)
        big_buf[t // TPC, t % TPC, :m] = xs[starts[t] : starts[t] + m]
    if nspill:
        sp_buf = np.zeros((N_CORES, nspill, sl, DIN), np.float32)
        sp_map = [[] for _ in range(N_CORES)]  # core -> [(slot, t, off, len)]
        for i, (t, off, ln) in enumerate(spills):
            core, slot = i % N_CORES, i // N_CORES
            sp_buf[core, slot, :ln] = xs[starts[t] + off : starts[t] + off + ln]
            sp_map[core].append((slot, t, off, ln))

    # per-(slot, stream) weight/bias/scale selection
    def slot_types(core):
        sts = [core * TPC + g for g in range(TPC)]
        if nspill:
            got = {slot: t for slot, t, _, _ in sp_map[core]}
            sts += [got.get(sidx, 0) for sidx in range(nspill)]
        return sts

    wsh_arr = np.ascontiguousarray(
        shared_w[0].reshape(2, 128, 2, 128)  # [k, p, c, q]
        .transpose(1, 2, 0, 3)  # [p, c, k, q]
        .reshape(128, 512)
        .astype(BF16)
    )

    # bias-fold deltas for back spill slots: x~ = x + b W^-1 (so the
    # bias-free merged ACT is exact); solved in float64 for stability
    delta = np.empty((NTYPES, NS, DIN), np.float64)
    for t in range(NTYPES):
        e0, e1 = top2[t]
        for sidx, (W, b) in enumerate(
            ((expert_w[e0], expert_b[e0]), (expert_w[e1], expert_b[e1]),
             (shared_w[0], shared_b[0]))
        ):
            delta[t, sidx] = np.linalg.solve(
                W.astype(np.float64).T, b.astype(np.float64)
            )

    in_maps = []
    for core in range(N_CORES):
        sts = slot_types(core)
        nslots = len(sts)
        w_sel = np.empty((nslots, 2, DIN, DOUT), np.float32)
        b_sel = np.empty((nslots, NS, DOUT), np.float32)
        s_sel = np.empty((nslots, 2), np.float32)
        for i, t in enumerate(sts):
            e0, e1 = top2[t]
            w_sel[i, 0], w_sel[i, 1] = expert_w[e0], expert_w[e1]
            b_sel[i, 0], b_sel[i, 1], b_sel[i, 2] = (
                expert_b[e0], expert_b[e1], shared_b[0],
            )
            s_sel[i] = wts[t]

        wb = (
            w_sel.reshape(nslots, 2, 2, 128, 2, 128)  # [i, s, k, p, c, q]
            .transpose(3, 0, 1, 4, 2, 5)  # [p, i, s, c, k, q]
            .reshape(128, nslots * WCOL)
            .astype(BF16)
        )
        bb = (
            b_sel.reshape(nslots, NS, 2, 128)  # [i, s, c, p]
            .transpose(3, 0, 1, 2)
            .reshape(128, nslots * NS * 2)
            .astype(np.float32)
        )
        sb_arr = np.broadcast_to(
            s_sel.reshape(1, nslots * 2), (128, nslots * 2)
        ).astype(np.float32)

        im = {
            "xtb": _xt_layout(big_buf[core]),
            "wsh": wsh_arr,
            "w": np.ascontiguousarray(wb),
            "bs": np.ascontiguousarray(
                np.concatenate([bb, sb_arr], axis=1).astype(np.float32)
            ),
        }
        if nspill:
            im["xtsf"] = _xt_layout(sp_buf[core][0:1])
        if nback:
            # [nback, sl, NS, 2, 128] -> [nback*128, NS*2*sl] with
            # col = (s*2+k)*sl + a, delta added per stream
            bk = np.empty((nback, sl, NS, DIN), np.float64)
            for j in range(nback):
                t = sts[TPC + 1 + j]
                bk[j] = sp_buf[core, 1 + j][:, None, :] + delta[t][None, :, :]
            im["xtsb"] = np.ascontiguousarray(
                bk.reshape(nback, sl, NS, 2, 128)  # [j, a, s, k, p]
                .transpose(0, 4, 2, 3, 1)  # [j, p, s, k, a]
            ).reshape(nback * 128, NS * 2 * sl).astype(BF16)
        in_maps.append(im)

    key = (TPC, nspill, sl)
    if key not in _compiled_cache:
        _compiled_cache[key] = _build_nc(TPC, nspill, sl)
    nc = _compiled_cache[key]

    kwargs = {}
    if _trace:
        kwargs["trace"] = True
        kwargs.update(_trace_kwargs or {})
    res = run_bass_kernel_spmd(nc, in_maps, core_ids=list(range(N_CORES)), **kwargs)

    # reassemble
    out_sorted = np.empty((N, DOUT), np.float32)
    for core in range(N_CORES):
        yb = res.results[core]["yb"].reshape(TPC, 2, 128, CAP)
        for g in range(TPC):
            t = core * TPC + g
            m = min(int(counts[t]), CAP)
            # [c, p, a] -> [a, c*128+p]
            blk = yb[g, :, :, :m].reshape(256, m).T
            out_sorted[starts[t] : starts[t] + m] = blk
        if nspill:
            ys = res.results[core]["ys"].reshape(nspill, 2, 128, sl)
            for slot, t, off, ln in sp_map[core]:
                blk = ys[slot, :, :, :ln].reshape(256, ln).T
                out_sorted[starts[t] + off : starts[t] + off + ln] = blk
    out = np.zeros((N, DOUT), np.float32)
    out[order] = out_sorted
    out = out.reshape(NB, NLOC, DOUT)

    if _trace:
        return out, res
    return out


# revision 56
# speedup vs baseline: 1.0171x; 1.0171x over previous
"""Trainium2 Bass kernel for the MoE routing layer (nn_MoELayer_20358144983731).

Strategy
--------
Routing depends only on the atom's type (32 types), and with top-2-of-8
routing each atom needs exactly 3 expert MLPs (2 routed + 1 shared) instead
of the reference's dense 9.  The gate is tiny, so it is computed on the host;
atoms are sorted by type and packed into fixed-capacity slots (CAP=2048
atoms; types larger than CAP get a small spill slot), distributed across the
8 NeuronCores.  Every atom of a type shares the same two routed experts and
scalar routing weights, so the whole device program is data-driven (weights /
biases / scales arrive as per-core input tensors) and a single SPMD program
runs on all 8 cores.

Per slot the device computes, transposed (z.T = [dout, atoms]) so the
dout-dim bias lands on partitions:
    y = w0*tanh(X W0 + b0) + w1*tanh(X W1 + b1) + tanh(X Ws + bs)

The kernel is ScalarE-bound (tanh ACT is scalar-engine-only, 1 elem/cyc/lane
plus ~650ns access-latency per instruction), so the structure keeps the
scalar queue saturated with nothing but back-to-back ACTs:
  - a dummy warm-up ACT triggers the tanh table load during the DMA phase
  - DMA is packet-dispatch-bound (~one descriptor per partition row per
    transfer), so the ramp minimizes wave-0 transfers across the three DMA
    rings (scalar/sync/gpsimd); big0's xt arrives as half-tiles and its
    first ACT is split in halves so the big stream starts as early as
    possible, while later xt/weight transfers are deferred into the slot
    loop where they queue behind output-DMA waits
  - one spill slot runs first (small ACTs warm the pipe while the big xt
    transfers land); remaining spill slots run last with their expert bias
    pre-folded into the spill atoms (x~ = x + b W^-1, solved on the host),
    which lets the 3 streams share one bias-free ACT and keeps the tail short
  - matmuls run in bf16 (weights stationary, atoms moving, N=512, fp32 PSUM
    accumulation, 4-bank tiles double buffered); the shared-expert weights
    load once and serve every slot's stream-2 matmuls
  - the 3-stream combine is two scalar_tensor_tensor ops on the vector
    engine (bf16 in / fp32 out); the final slot's last ACT and combine are
    split in halves and its output partitioned over the then-idle DMA rings
    so the tail drains fast
"""

import sys

import numpy as np

try:
    import concourse  # noqa: F401
except ImportError:  # grading container path
    sys.path.insert(0, "/opt/trn_rl_repo")

import ml_dtypes

import concourse.bacc as bacc
import concourse.mybir as mybir
import concourse.tile as tile
from concourse.bass_utils import run_bass_kernel_spmd

NB, NLOC = 4, 16384
DIN, DOUT = 256, 256
NTYPES = 32
N_CORES = 8
NS = 3  # streams: routed expert 0, routed expert 1, shared expert
CAP = 2048  # big-slot capacity (4 PSUM banks at fp32)
BF16 = ml_dtypes.bfloat16
WCOL = 2 * 2 * 2 * 128  # routed weight columns per slot (2 streams)

_compiled_cache = {}


def _build_nc(nbig, nspill, sl):
    """Build + compile the SPMD Tile kernel.

    nbig:   number of CAP-length slots per core
    nspill: number of spill slots per core (0 = none); the first runs
            before the big slots (bias via ACT), the rest run after them
            (bias pre-folded into x, single ACT per dout-half)
    sl:     spill slot length (multiple of 128)
    """
    f32 = mybir.dt.float32
    bf16 = mybir.dt.bfloat16
    Tanh = mybir.ActivationFunctionType.Tanh
    mult = mybir.AluOpType.mult
    add = mybir.AluOpType.add

    nslots = nbig + nspill
    nback = max(nspill - 1, 0)

    nc = bacc.Bacc("TRN2", target_bir_lowering=False, debug=False)
    xtb_d = nc.dram_tensor("xtb", [nbig * 128, 2 * CAP], bf16, kind="ExternalInput")
    if nspill:
        xtsf_d = nc.dram_tensor("xtsf", [128, 2 * sl], bf16, kind="ExternalInput")
    if nback:
        xtsb_d = nc.dram_tensor(
            "xtsb", [nback * 128, NS * 2 * sl], bf16, kind="ExternalInput"
        )
    wsh_d = nc.dram_tensor("wsh", [128, 512], bf16, kind="ExternalInput")
    w_d = nc.dram_tensor("w", [128, nslots * WCOL], bf16, kind="ExternalInput")
    b_d = nc.dram_tensor("b", [128, nslots * NS * 2], f32, kind="ExternalInput")
    s_d = nc.dram_tensor("s", [128, nslots * 2], f32, kind="ExternalInput")
    yb_d = nc.dram_tensor("yb", [nbig * 2 * 128, CAP], f32, kind="ExternalOutput")
    if nspill:
        ys_d = nc.dram_tensor("ys", [nspill * 2 * 128, sl], f32, kind="ExternalOutput")

    with tile.TileContext(nc) as tc:
        with (
            tc.tile_pool(name="const", bufs=1) as constp,
            tc.tile_pool(name="t", bufs=3) as tp,
            tc.tile_pool(name="y", bufs=3) as yp,
            tc.tile_pool(name="ps", bufs=2, space="PSUM") as psp,
        ):
            # dummy ACT so the tanh table load happens during the DMA phase
            dm_in = constp.tile([128, 1], f32, name="dmi")
            dm_out = constp.tile([128, 1], f32, name="dmo")
            nc.vector.memset(dm_in, 0.0)
            nc.scalar.activation(dm_out, dm_in, Tanh, bias=dm_in, scale=1.0)

            # ---- DMA issue: spread across the three DMA-capable queues -
            # scalar (idle until first ACT): wsh, xtb0
            # sync: spill-front xt, b, s, xtb1..3, spill-back xt; later y (c=1)
            # gpsimd: all slot weights; later y (c=0)
            # big0's xt comes as two half-tiles (k=0 / k=1), each striped
            # over two DMA channels, so its matmuls can start ~11us in;
            # other big slots use one whole tile each on the sync channel
            xt0_k = [
                constp.tile([128, CAP], bf16, name=f"xt0k{k}") for k in range(2)
            ]
            xt_big = [None] + [
                constp.tile([128, 2 * CAP], bf16, name=f"xtb{i}")
                for i in range(1, nbig)
            ]
            if nspill:
                xt_f = constp.tile([128, 2 * sl], bf16, name="xtf")
                nc.sync.dma_start(out=xt_f, in_=xtsf_d.ap())
            wsh = constp.tile([128, 512], bf16, name="wsh")
            nc.scalar.dma_start(out=wsh, in_=wsh_d.ap())
            b_sb = constp.tile([128, nslots * NS * 2], f32)
            nc.sync.dma_start(out=b_sb, in_=b_d.ap())
            s_sb = constp.tile([128, nslots * 2], f32)
            nc.sync.dma_start(out=s_sb, in_=s_d.ap())
            w_sl = {
                si: constp.tile([128, WCOL], bf16, name=f"w{si}")
                for si in range(nslots)
            }

            def w_dma(engine, si):
                engine.dma_start(
                    out=w_sl[si], in_=w_d.ap()[:, si * WCOL : (si + 1) * WCOL]
                )

            # DMA is packet-dispatch-bound (~1 descriptor per partition row),
            # so the ramp is bounded by the number of wave-0 transfers; big0's
            # xt comes as two half-tiles (one DMA each) and everything not
            # needed early is deferred into the slot loop so it queues behind
            # output-DMA waits.
            half = CAP // 2
            if nspill:
                w_dma(nc.gpsimd, nbig)  # front spill slot weights
            nc.scalar.dma_start(
                out=xt0_k[0][:, :half], in_=xtb_d.ap()[0:128, 0:half]
            )
            nc.gpsimd.dma_start(
                out=xt0_k[1][:, :half], in_=xtb_d.ap()[0:128, CAP : CAP + half]
            )
            nc.sync.dma_start(
                out=xt0_k[0][:, half:], in_=xtb_d.ap()[0:128, half:CAP]
            )
            nc.sync.dma_start(
                out=xt0_k[1][:, half:], in_=xtb_d.ap()[0:128, CAP + half : 2 * CAP]
            )
            w_dma(nc.gpsimd, 0)
            xt_b = [
                constp.tile([128, NS * 2 * sl], bf16, name=f"xtbk{j}")
                for j in range(nback)
            ]

            # deferred transfers: issued inside the slot loop right after a
            # blocking output DMA so their packets don't compete with the
            # startup burst on the shared DMA engines
            deferred_sync = [
                (lambda i=i: nc.sync.dma_start(
                    out=xt_big[i], in_=xtb_d.ap()[i * 128 : (i + 1) * 128, :]
                )) for i in range(1, nbig)
            ] + [
                (lambda j=j: w_dma(nc.sync, nbig + 1 + j)) for j in range(nback)
            ] + [
                (lambda j=j: nc.sync.dma_start(
                    out=xt_b[j], in_=xtsb_d.ap()[j * 128 : (j + 1) * 128, :]
                )) for j in range(nback)
            ]
            deferred_gp = [
                (lambda i=i: w_dma(nc.gpsimd, i)) for i in range(1, nbig)
            ]

            # ---- slot schedule: spill-front, bigs, spill-backs ---------
            sched = []
            if nspill:
                sched.append(("front", nbig))
            sched += [("big", i) for i in range(nbig)]
            sched += [("back", nbig + 1 + j) for j in range(nback)]

            def combine(si, t_sb, slen, dst_d, yrow, pieces, tail=False,
                        eng=None, last=False):
                eng = eng or nc.vector
                ycomb = yp.tile([128, slen], f32, tag="yc", name="yc")
                for pi, (h0, h1) in enumerate(pieces):
                    ya = yp.tile([128, slen], f32, tag="ya", name="ya")
                    eng.scalar_tensor_tensor(
                        ya[:, : h1 - h0],
                        t_sb[:, h0:h1],
                        s_sb[:, si * 2 : si * 2 + 1],
                        t_sb[:, 2 * slen + h0 : 2 * slen + h1],
                        mult,
                        add,
                    )
                    eng.scalar_tensor_tensor(
                        ycomb[:, h0:h1],
                        t_sb[:, slen + h0 : slen + h1],
                        s_sb[:, si * 2 + 1 : si * 2 + 2],
                        ya[:, : h1 - h0],
                        mult,
                        add,
                    )
                    if last:
                        # very last outputs: halve the packet count per ring
                        # by splitting each piece over two idle rings
                        for q, (r0, r1) in zip(
                            (nc.scalar, nc.gpsimd), ((0, 64), (64, 128))
                        ):
                            q.dma_start(
                                out=dst_d.ap()[yrow + r0 : yrow + r1, h0:h1],
                                in_=ycomb[r0:r1, h0:h1],
                            )
                        continue
                    if tail:
                        # scalar's DMA channel is idle once ACTs are done,
                        # but it is the slow one: give it the earliest piece
                        q = (nc.scalar, nc.gpsimd, nc.sync, nc.gpsimd)[pi % 4]
                    else:
                        q = nc.gpsimd if (yrow // 128) % 2 == 0 else nc.sync
                    q.dma_start(
                        out=dst_d.ap()[yrow : yrow + 128, h0:h1],
                        in_=ycomb[:, h0:h1],
                    )

            for oi, (kind, si) in enumerate(sched):
                is_last_big = kind == "big" and si == nbig - 1
                if kind == "big":
                    slen, xt_sb, dst_d, r0 = CAP, xt_big[si], yb_d, si
                    if si == 0:
                        rhs = lambda k, a0, al: xt0_k[k][:, a0 : a0 + al]
                    else:
                        rhs = (lambda xt: lambda k, a0, al:
                               xt[:, k * CAP + a0 : k * CAP + a0 + al])(xt_sb)
                elif kind == "front":
                    slen, xt_sb, dst_d, r0 = sl, xt_f, ys_d, 0
                    rhs = lambda k, a0, al: xt_f[:, k * sl + a0 : k * sl + a0 + al]
                else:
                    slen, xt_sb, dst_d, r0 = sl, xt_b[si - nbig - 1], ys_d, si - nbig

                for c in range(2):
                    yrow = (r0 * 2 + c) * 128
                    if kind == "back":
                        # 3 streams in one bias-free ACT (bias folded into x)
                        ps = psp.tile([128, NS * slen], f32, tag="ps", name="ps")
                        first = True
                        for s in (2, 0, 1):
                            for k in range(2):
                                if s == 2:
                                    lhsT = wsh[:, (c * 2 + k) * 128 : (c * 2 + k + 1) * 128]
                                else:
                                    blk = (s * 2 + c) * 2 + k
                                    lhsT = w_sl[si][:, blk * 128 : (blk + 1) * 128]
                                nc.tensor.matmul(
                                    ps[:, s * slen : (s + 1) * slen],
                                    lhsT,
                                    xt_sb[:, (s * 2 + k) * slen : (s * 2 + k + 1) * slen],
                                    start=first,
                                    stop=(s == 1 and k == 1),
                                    skip_group_check=True,
                                )
                                first = False
                        t_sb = tp.tile([128, NS * slen], bf16, tag="t", name="t")
                        nc.scalar.activation(t_sb, ps, Tanh, bias=dm_in, scale=1.0)
                        # t layout: stream s at [s*slen,(s+1)*slen) but ACT wrote
                        # psum order (2,0,1)->(0,1,2)? ps col s*slen+a == t col
                        # s*slen+a, so layout already stream-major.
                        combine(si, t_sb, slen, dst_d, yrow, ((0, slen),))
                        continue
                    t_sb = tp.tile([128, NS * slen], bf16, tag="t", name="t")
                    # stream order (2, 0, 1): combine #1 needs t2+t0, so the
                    # tail combine only waits on the last stream's ACT
                    for s in (2, 0, 1):
                        bcol = (si * NS + s) * 2 + c
                        ps = psp.tile([128, slen], f32, tag="ps", name="ps")
                        if kind == "big" and si == 0 and c == 0 and s == 2 \
                                and slen > 512:
                            # first big ACT split in halves: starts once only
                            # half of big0's xt has landed
                            hh = slen // 2
                            for h0 in (0, hh):
                                for k in range(2):
                                    lhsT = wsh[:, (c * 2 + k) * 128 : (c * 2 + k + 1) * 128]
                                    for a0 in range(h0, h0 + hh, 512):
                                        nc.tensor.matmul(
                                            ps[:, a0 : a0 + 512],
                                            lhsT,
                                            rhs(k, a0, 512),
                                            start=(k == 0),
                                            stop=(k == 1),
                                        )
                                nc.scalar.activation(
                                    t_sb[:, s * slen + h0 : s * slen + h0 + hh],
                                    ps[:, h0 : h0 + hh],
                                    Tanh,
                                    bias=b_sb[:, bcol : bcol + 1],
                                    scale=1.0,
                                )
                            continue
                        for k in range(2):
                            if s == 2:
                                lhsT = wsh[:, (c * 2 + k) * 128 : (c * 2 + k + 1) * 128]
                            else:
                                blk = (s * 2 + c) * 2 + k
                                lhsT = w_sl[si][:, blk * 128 : (blk + 1) * 128]
                            for a0 in range(0, slen, 512):
                                al = min(512, slen - a0)
                                nc.tensor.matmul(
                                    ps[:, a0 : a0 + al],
                                    lhsT,
                                    rhs(k, a0, al),
                                    start=(k == 0),
                                    stop=(k == 1),
                                )
                        # tanh + per-partition bias, PSUM -> SBUF (bf16)
                        if s == 1 and is_last_big and c == 1 and slen > 512:
                            # split the final ACT so the combine + output DMA
                            # pipeline starts while its second half runs
                            hh = slen // 2
                            for p0 in (0, hh):
                                nc.scalar.activation(
                                    t_sb[:, slen + p0 : slen + p0 + hh],
                                    ps[:, p0 : p0 + hh],
                                    Tanh,
                                    bias=b_sb[:, bcol : bcol + 1],
                                    scale=1.0,
                                )
                        else:
                            nc.scalar.activation(
                                t_sb[:, s * slen : (s + 1) * slen],
                                ps,
                                Tanh,
                                bias=b_sb[:, bcol : bcol + 1],
                                scale=1.0,
                            )
                    if slen > 512 and is_last_big:
                        hh = slen // 2
                        pieces = ((0, hh), (hh, slen))
                    else:
                        pieces = ((0, slen),)
                    combine(si, t_sb, slen, dst_d, yrow, pieces,
                            tail=is_last_big)
                    if kind == "big":
                        # deferred transfers ride behind the y DMA just
                        # issued on their queue (c=0 -> gpsimd, c=1 -> sync)
                        if c == 0:
                            if deferred_gp:
                                deferred_gp.pop(0)()
                        else:
                            for _ in range(2):
                                if deferred_sync:
                                    deferred_sync.pop(0)()

    nc.compile()
    return nc


def _host_route(type_embeddings, gate_w):
    """Gate on host: per-type top-2 experts + softmax weights (tiny)."""
    logits = type_embeddings.astype(np.float32) @ gate_w.astype(np.float32)
    top2 = np.argsort(-logits, axis=1, kind="stable")[:, :2]
    tv = np.take_along_axis(logits, top2, axis=1)
    e = np.exp(tv - tv.max(axis=1, keepdims=True))
    wts = e / e.sum(axis=1, keepdims=True)
    return top2, wts


def _xt_layout(buf):
    """[nslots, slen, 256] fp32 -> [nslots*128, 2*slen] bf16 with
    row = slot*128 + p, col = k*slen + a, value = buf[slot, a, k*128+p]."""
    ns, slen, _ = buf.shape
    return np.ascontiguousarray(
        buf.reshape(ns, slen, 2, 128).transpose(0, 3, 2, 1)  # [slot, p, k, a]
    ).reshape(ns * 128, 2 * slen).astype(BF16)


def kernel(x, type_embeddings, atom_types, gate_w, expert_w, expert_b,
           shared_w, shared_b, _trace=False, _trace_kwargs=None):
    x = np.asarray(x, dtype=np.float32)
    type_embeddings = np.asarray(type_embeddings, dtype=np.float32)
    atom_types = np.asarray(atom_types)
    gate_w = np.asarray(gate_w, dtype=np.float32)
    expert_w = np.asarray(expert_w, dtype=np.float32)
    expert_b = np.asarray(expert_b, dtype=np.float32)
    shared_w = np.asarray(shared_w, dtype=np.float32)
    shared_b = np.asarray(shared_b, dtype=np.float32)

    top2, wts = _host_route(type_embeddings, gate_w)

    flat_t = atom_types.reshape(-1).astype(np.int64)
    N = flat_t.size
    order = np.argsort(flat_t, kind="stable")
    counts = np.bincount(flat_t, minlength=NTYPES)
    starts = np.zeros(NTYPES + 1, np.int64)
    starts[1:] = np.cumsum(counts)
    xs = x.reshape(N, DIN)[order]

    # pieces: per type a main piece (<= CAP rows) + spill pieces
    # big slot: type t -> core t // (NTYPES//N_CORES), slot t % (..)
    TPC = NTYPES // N_CORES  # big slots per core = 4
    spills = []  # (type, offset_in_type, length)
    for t in range(NTYPES):
        off = CAP
        while off < counts[t]:
            spills.append((t, off, min(CAP, counts[t] - off)))
            off += CAP
    nspill = (len(spills) + N_CORES - 1) // N_CORES
    max_spill = max((ln for _, _, ln in spills), default=0)
    sl = max(((max_spill + 127) // 128) * 128, 128) if nspill else 0
    nback = max(nspill - 1, 0)

    big_buf = np.zeros((N_CORES, TPC, CAP, DIN), np.float32)
    for t in range(NTYPES):
        m = min(int(counts[t]), CAP)
        big_buf[t // TPC, t % TPC, :m] = xs[starts[t] : starts[t] + m]
    if nspill:
        sp_buf = np.zeros((N_CORES, nspill, sl, DIN), np.float32)
        sp_map = [[] for _ in range(N_CORES)]  # core -> [(slot, t, off, len)]
        for i, (t, off, ln) in enumerate(spills):
            core, slot = i % N_CORES, i // N_CORES
            sp_buf[core, slot, :ln] = xs[starts[t] + off : starts[t] + off + ln]
            sp_map[core].append((slot, t, off, ln))

    # per-(slot, stream) weight/bias/scale selection
    def slot_types(core):
        sts = [core * TPC + g for g in range(TPC)]
        if nspill:
            got = {slot: t for slot, t, _, _ in sp_map[core]}
            sts += [got.get(sidx, 0) for sidx in range(nspill)]
        return sts

    wsh_arr = np.ascontiguousarray(
        shared_w[0].reshape(2, 128, 2, 128)  # [k, p, c, q]
        .transpose(1, 2, 0, 3)  # [p, c, k, q]
        .reshape(128, 512)
        .astype(BF16)
    )

    # bias-fold deltas for back spill slots: x~ = x + b W^-1 (so the
    # bias-free merged ACT is exact); solved in float64 for stability
    delta = np.empty((NTYPES, NS, DIN), np.float64)
    for t in range(NTYPES):
        e0, e1 = top2[t]
        for sidx, (W, b) in enumerate(
            ((expert_w[e0], expert_b[e0]), (expert_w[e1], expert_b[e1]),
             (shared_w[0], shared_b[0]))
        ):
            delta[t, sidx] = np.linalg.solve(
                W.astype(np.float64).T, b.astype(np.float64)
            )

    in_maps = []
    for core in range(N_CORES):
        sts = slot_types(core)
        nslots = len(sts)
        w_sel = np.empty((nslots, 2, DIN, DOUT), np.float32)
        b_sel = np.empty((nslots, NS, DOUT), np.float32)
        s_sel = np.empty((nslots, 2), np.float32)
        for i, t in enumerate(sts):
            e0, e1 = top2[t]
            w_sel[i, 0], w_sel[i, 1] = expert_w[e0], expert_w[e1]
            b_sel[i, 0], b_sel[i, 1], b_sel[i, 2] = (
                expert_b[e0], expert_b[e1], shared_b[0],
            )
            s_sel[i] = wts[t]

        wb = (
            w_sel.reshape(nslots, 2, 2, 128, 2, 128)  # [i, s, k, p, c, q]
            .transpose(3, 0, 1, 4, 2, 5)  # [p, i, s, c, k, q]
            .reshape(128, nslots * WCOL)
            .astype(BF16)
        )
        bb = (
            b_sel.reshape(nslots, NS, 2, 128)  # [i, s, c, p]
            .transpose(3, 0, 1, 2)
            .reshape(128, nslots * NS * 2)
            .astype(np.float32)
        )
        sb_arr = np.broadcast_to(
            s_sel.reshape(1, nslots * 2), (128, nslots * 2)
        ).astype(np.float32)

        im = {
            "xtb": _xt_layout(big_buf[core]),
            "wsh": wsh_arr,
            "w": np.ascontiguousarray(wb),
            "b": np.ascontiguousarray(bb),
            "s": np.ascontiguousarray(sb_arr),
        }
        if nspill:
            im["xtsf"] = _xt_layout(sp_buf[core][0:1])
        if nback:
            # [nback, sl, NS, 2, 128] -> [nback*128, NS*2*sl] with
            # col = (s*2+k)*sl + a, delta added per stream
            bk = np.empty((nback, sl, NS, DIN), np.float64)
            for j in range(nback):
                t = sts[TPC + 1 + j]
                bk[j] = sp_buf[core, 1 + j][:, None, :] + delta[t][None, :, :]
            im["xtsb"] = np.ascontiguousarray(
                bk.reshape(nback, sl, NS, 2, 128)  # [j, a, s, k, p]
                .transpose(0, 4, 2, 3, 1)  # [j, p, s, k, a]
            ).reshape(nback * 128, NS * 2 * sl).astype(BF16)
        in_maps.append(im)

    key = (TPC, nspill, sl)
    if key not in _compiled_cache:
        _compiled_cache[key] = _build_nc(TPC, nspill, sl)
    nc = _compiled_cache[key]

    kwargs = {}
    if _trace:
        kwargs["trace"] = True
        kwargs.update(_trace_kwargs or {})
    res = run_bass_kernel_spmd(nc, in_maps, core_ids=list(range(N_CORES)), **kwargs)

    # reassemble
    out_sorted = np.empty((N, DOUT), np.float32)
    for core in range(N_CORES):
        yb = res.results[core]["yb"].reshape(TPC, 2, 128, CAP)
        for g in range(TPC):
            t = core * TPC + g
            m = min(int(counts[t]), CAP)
            # [c, p, a] -> [a, c*128+p]
            blk = yb[g, :, :, :m].reshape(256, m).T
            out_sorted[starts[t] : starts[t] + m] = blk
        if nspill:
            ys = res.results[core]["ys"].reshape(nspill, 2, 128, sl)
            for slot, t, off, ln in sp_map[core]:
                blk = ys[slot, :, :, :ln].reshape(256, ln).T
                out_sorted[starts[t] + off : starts[t] + off + ln] = blk
    out = np.zeros((N, DOUT), np.float32)
    out[order] = out_sorted
    out = out.reshape(NB, NLOC, DOUT)

    if _trace:
        return out, res
    return out
